# revision 5
# baseline (speedup 1.0000x reference)
"""NeuralKB retrieval kernel v2 for Trainium2 (Bass/Tile), 8-core SPMD.

Math (per score s in {sp, po}, batch b, entity n):
  d2min[s,b,n] = q2part[s,b] + e2[n] + min_f (f2[f] - 2*A[s,b,f] - 2*C[s,n,f])
  out[s,b,n]   = min(exp(-0.5 * (e2 - M)), 1),  M = max_f (2C + 2A - f2 - q2part)

Sharding: data-parallel over N (500 entities/core, padded to 512).
Layout: f on partitions (32 chunks of 128), n on free (512).

v2 vs v1:
- Loop-invariant work (fact transposes/casts, f2, W) hoisted out of the
  steady-state chunk loop; in the one-shot build it is interleaved per
  group of 8 chunks so stage-1 starts after ~1/4 of the input prep.
- W kept both as columns (W_all f32: per-partition scalars for lane adds)
  and as bf16 rows (Wrows: PE fusion below).
- 'p' sections: PE computes C + W in one PSUM tile per section via the
  C-part matmul plus a rank-16 matmul Wrows[:,chunk]^T @ onehot[sec]
  (one-hot column block selects that section's W row, broadcast over n);
  Pool then max-accumulates straight from PSUM. No lane add at all.
- Remaining sections' adds split DVE (bf16 cast, 4x mode, 194ns) /
  ACT (PSUM + bias, 612ns) / Pool (PSUM direct, 427ns); maxes split
  batched-DVE (266/512-block) / Pool (427).
- acc/out kept in 'slot' order (batched-DVE group contiguous);
  assemble() un-permutes on the host for free.
"""

import numpy as np

import concourse.bass as bass
import concourse.tile as tile
from concourse import bacc, mybir
from concourse import bass_utils
from concourse.masks import make_identity
from concourse.bass_isa import ReduceOp

F32 = mybir.dt.float32
BF16 = mybir.dt.bfloat16
AF = mybir.ActivationFunctionType
ALU = mybir.AluOpType

B = 8
E = 100
F = 4000
FP = 4096
NCHUNK = FP // 128  # 32
GROUPS = 4
GCH = NCHUNK // GROUPS  # 8
NCORE = 512
NSEC = 16  # (score, b) sections; id = score*8 + b
NEG_BIG = -3.0e38

# Per-section engine plan (index = score*8+b).
# add: 'p' PE-fused, 'v' DVE (bf16 cast), 'a' ACT (PSUM+bias), 'g' Pool (PSUM)
# max: 'd' batched DVE tt-max, 'g' per-section Pool tt-max ('p' adds => 'g')
# 'n' = anchor: ACT bias-add from PSUM writing bf16 xt; v/g sections add
# (w_sec - w_anchor) to the anchor's xt slice (bf16, no PSUM cast needed).
# Pool supports only tensor_scalar (no TensorTensor) on TRN2 HW, and cannot
# touch PSUM; so all max-accumulates are one batched DVE tensor_tensor and
# Pool only does bf16 delta-adds off each score's anchor section.
ADD_ENGINES = list("naaggggg" "naaagggg")
MAX_ENGINES = list("dddddddd" "dddddddd")


def section_order(max_engines=None):
    max_engines = max_engines or MAX_ENGINES
    d_secs = [s for s in range(NSEC) if max_engines[s] == "d"]
    h_secs = [s for s in range(NSEC) if max_engines[s] == "h"]
    g_secs = [s for s in range(NSEC) if max_engines[s] not in ("d", "h")]
    return d_secs + h_secs + g_secs


def build_bass(add_engines=None, max_engines=None, repeat=1,
               cbufs=5, pbufs=3, xbufs=3, csbufs=3, vcast="a", debug=False):
    add_engines = add_engines or ADD_ENGINES
    max_engines = max_engines or MAX_ENGINES
    nc = bacc.Bacc("TRN2", target_bir_lowering=False, debug=False, num_devices=8)

    f_rel = nc.dram_tensor("f_rel", [FP, E], F32, kind="ExternalInput")
    f_a1 = nc.dram_tensor("f_a1", [FP, E], F32, kind="ExternalInput")
    f_a2 = nc.dram_tensor("f_a2", [FP, E], F32, kind="ExternalInput")
    ent = nc.dram_tensor("ent", [NCORE, E], F32, kind="ExternalInput")
    rel = nc.dram_tensor("rel", [B, E], F32, kind="ExternalInput")
    a1 = nc.dram_tensor("a1", [B, E], F32, kind="ExternalInput")
    a2 = nc.dram_tensor("a2", [B, E], F32, kind="ExternalInput")
    # out rows are in 'slot' order; assemble() un-permutes host-side
    out = nc.dram_tensor("out", [NSEC, NCORE], F32, kind="ExternalOutput")
    dbg = None
    if debug:
        dbg = {
            "dbg_acc": nc.dram_tensor("dbg_acc", [128, NSEC * NCORE], BF16, kind="ExternalOutput"),
            "dbg_m16": nc.dram_tensor("dbg_m16", [NSEC, NCORE], BF16, kind="ExternalOutput"),
            "dbg_e2": nc.dram_tensor("dbg_e2", [1, NCORE], F32, kind="ExternalOutput"),
            "dbg_w": nc.dram_tensor("dbg_w", [128, NCHUNK * NSEC], F32, kind="ExternalOutput"),
            "dbg_wd": nc.dram_tensor("dbg_wd", [128, NCHUNK * NSEC], F32, kind="ExternalOutput"),
            "dbg_f2": nc.dram_tensor("dbg_f2", [128, NCHUNK], F32, kind="ExternalOutput"),
        }

    with tile.TileContext(nc) as tc:
        _kernel_body(nc, tc, f_rel, f_a1, f_a2, ent, rel, a1, a2, out,
                     add_engines, max_engines, repeat, cbufs, pbufs, xbufs,
                     csbufs, vcast, dbg)
    nc.compile()
    return nc


def _kernel_body(nc, tc, f_rel, f_a1, f_a2, ent, rel, a1, a2, out,
                 add_engines, max_engines, repeat, cbufs, pbufs, xbufs,
                 csbufs, vcast, dbg=None):
    secs = list(range(NSEC))
    order = section_order(max_engines)
    d_secs = [s for s in secs if max_engines[s] == "d"]
    h_secs = [s for s in secs if max_engines[s] == "h"]
    assert len(h_secs) <= 1, "at most one half-split max section"
    g_secs = [s for s in secs if max_engines[s] not in ("d", "h")]
    slot = {sec: i for i, sec in enumerate(order)}
    p_secs = [s for s in secs if add_engines[s] == "p"]
    for s in p_secs:
        assert max_engines[s] == "g", "p-sections read PSUM; use Pool max"
    anchor = {}
    for s in secs:
        if add_engines[s] == "n":
            sc = s // B
            assert sc not in anchor, "one anchor per score"
            anchor[sc] = s
    for s in secs:
        if add_engines[s] in ("v", "g"):
            assert s // B in anchor, "delta sections need an anchor in their score"

    with (
        tc.tile_pool(name="const", bufs=1) as const_pool,
        tc.tile_pool(name="factT", bufs=1) as factT_pool,
        tc.tile_pool(name="acc", bufs=1) as acc_pool,
        tc.tile_pool(name="small", bufs=1) as small_pool,
        tc.tile_pool(name="fin", bufs=1) as fin_pool,
        tc.tile_pool(name="nat", bufs=1) as nat_pool,
        tc.tile_pool(name="sq", bufs=3) as sq_pool,
    ):
        st = {}  # shared tiles
        st["anchor"] = anchor
        st["ident"] = const_pool.tile([128, 128], F32, name="ident")
        make_identity(nc, st["ident"][:])

        frelT = factT_pool.tile([101, FP], BF16, tag="frelT")
        st["frelT"] = frelT
        st["fa1T"] = factT_pool.tile([100, FP], BF16, name="fa1T")
        st["fa2T"] = factT_pool.tile([100, FP], BF16, name="fa2T")
        onesrowF = small_pool.tile([1, FP], BF16, tag="onesrowF")
        nc.gpsimd.memset(onesrowF[:], 1.0)
        nc.sync.dma_start(frelT[100:101, :], onesrowF[:])

        st["W_all"] = const_pool.tile([128, NCHUNK * NSEC], F32, name="W_all")
        st["W_del"] = const_pool.tile([128, NCHUNK * NSEC], F32, name="W_del")
        st["Wrows"] = const_pool.tile([NSEC, NCHUNK * 128], BF16, name="Wrows")
        onehots = const_pool.tile([NSEC, NSEC * NCORE], BF16, tag="onehots")
        st["onehots"] = onehots
        nc.gpsimd.memset(onehots[:], 0.0)
        for sec in p_secs:
            nc.sync.dma_start(
                onehots[sec : sec + 1, sec * NCORE : (sec + 1) * NCORE],
                onesrowF[0:1, :NCORE],
            )
        st["ent2T"] = const_pool.tile([100, NCORE], BF16, name="ent2T")
        st["e2row"] = small_pool.tile([1, NCORE], F32, name="e2row")
        acc_all = acc_pool.tile([128, NSEC * NCORE], BF16, tag="acc_all")
        st["acc_all"] = acc_all
        nc.gpsimd.memset(acc_all[:], NEG_BIG)
        st["f2cols"] = small_pool.tile([128, NCHUNK], F32, name="f2cols")

        with (
            tc.tile_pool(name="csb", bufs=csbufs) as csb_pool,
            tc.tile_pool(name="xall", bufs=xbufs) as xall_pool,
            tc.tile_pool(name="cpsum", bufs=cbufs, space="PSUM") as cpsum_pool,
            tc.tile_pool(name="ppsum", bufs=pbufs, space="PSUM") as ppsum_pool,
        ):
            import contextlib

            _prologue_head(nc, st, ent, rel, a1, a2, cpsum_pool, small_pool)
            nats = _issue_fact_loads(nc, nat_pool, f_rel, f_a1, f_a2)

            def stage1(chunks):
                _stage1(nc, add_engines, max_engines, slot, d_secs, h_secs,
                        g_secs, p_secs, anchor, st, chunks, csb_pool,
                        xall_pool, cpsum_pool, ppsum_pool, vcast)

            if repeat == 1:
                for g in range(GROUPS):
                    _prologue_group(nc, st, nats, g, sq_pool, cpsum_pool,
                                    ppsum_pool)
                    stage1(range(g * GCH, (g + 1) * GCH))
            else:
                for g in range(GROUPS):
                    _prologue_group(nc, st, nats, g, sq_pool, cpsum_pool,
                                    ppsum_pool)
                if isinstance(repeat, str):  # "uN" = static unroll N copies
                    for _ in range(int(repeat[1:])):
                        stage1(range(NCHUNK))
                elif isinstance(repeat, tuple):  # (R, U): For_i(R) x U unrolled
                    rr, uu = repeat
                    with tc.For_i(0, rr, 1):
                        for _ in range(uu):
                            stage1(range(NCHUNK))
                else:
                    with tc.For_i(0, repeat, 1):
                        stage1(range(NCHUNK))

        # ---------------- finals (slot order) ----------------------------
        if dbg is not None:
            nc.sync.dma_start(dbg["dbg_acc"].ap(), acc_all[:])
            nc.sync.dma_start(dbg["dbg_e2"].ap(), st["e2row"][:])
            nc.sync.dma_start(dbg["dbg_w"].ap(), st["W_all"][:])
            nc.sync.dma_start(dbg["dbg_wd"].ap(), st["W_del"][:])
            nc.sync.dma_start(dbg["dbg_f2"].ap(), st["f2cols"][:])
        accmax = acc_pool.tile([128, NSEC * NCORE], BF16, tag="accmax")
        m16 = fin_pool.tile([NSEC, NCORE], BF16, tag="m16")
        nc.gpsimd.partition_all_reduce(accmax[:], acc_all[:], 128, ReduceOp.max)
        for i in range(NSEC):
            nc.sync.dma_start(
                m16[i : i + 1, :], accmax[0:1, i * NCORE : (i + 1) * NCORE]
            )
        if dbg is not None:
            nc.sync.dma_start(dbg["dbg_m16"].ap(), m16[:])
        e2rep = fin_pool.tile([NSEC, NCORE], F32, tag="e2rep")
        nc.gpsimd.partition_broadcast(e2rep[:], st["e2row"][:])
        sub16 = fin_pool.tile([NSEC, NCORE], F32, tag="sub16")
        nc.vector.tensor_tensor(sub16[:], e2rep[:], m16[:], op=ALU.subtract)
        exp16 = fin_pool.tile([NSEC, NCORE], F32, tag="exp16")
        nc.scalar.activation(exp16[:], sub16[:], AF.Exp, scale=-0.5)
        out16 = fin_pool.tile([NSEC, NCORE], F32, tag="out16")
        nc.vector.tensor_scalar(
            out=out16[:], in0=exp16[:], scalar1=1.0, scalar2=None, op0=ALU.min
        )
        nc.sync.dma_start(out.ap(), out16[:])


def _prologue_head(nc, st, ent, rel, a1, a2, tpsum_pool, small_pool):
    ident = st["ident"]
    relmov = small_pool.tile([101, 2 * B], BF16, tag="relmov")
    a1mov = small_pool.tile([100, 2 * B], BF16, tag="a1mov")
    a2mov = small_pool.tile([100, 2 * B], BF16, tag="a2mov")
    st["relmov"], st["a1mov"], st["a2mov"] = relmov, a1mov, a2mov
    nc.gpsimd.memset(a1mov[:, B : 2 * B], 0.0)
    nc.gpsimd.memset(a2mov[:, 0:B], 0.0)

    qn = {}
    for name, dram in (("rel", rel), ("a1", a1), ("a2", a2)):
        qt = small_pool.tile([B, E], F32, tag=f"q_{name}")
        nc.sync.dma_start(qt[:], dram.ap())
        qn[name] = qt
    for name, dsts in (
        ("rel", ((relmov, 0), (relmov, B))),
        ("a1", ((a1mov, 0),)),
        ("a2", ((a2mov, B),)),
    ):
        tp = tpsum_pool.tile([128, NCORE], F32, tag="cp")
        nc.tensor.transpose(tp[:E, :B], qn[name][:], ident[:B, :B])
        for dst, coff in dsts:
            nc.scalar.activation(
                dst[0:100, coff : coff + B], tp[:E, :B], AF.Copy, scale=2.0
            )
    sqs = {}
    for name in ("rel", "a1", "a2"):
        sq = small_pool.tile([B, E], F32, tag=f"qsq_{name}")
        nc.scalar.activation(sq[:], qn[name][:], AF.Square)
        r = small_pool.tile([B, 1], F32, tag=f"qr_{name}")
        nc.vector.tensor_reduce(r[:], sq[:], axis=mybir.AxisListType.X, op=ALU.add)
        sqs[name] = r
    q2both = small_pool.tile([B, 2], F32, tag="q2both")
    nc.vector.tensor_tensor(q2both[:, 0:1], sqs["rel"][:], sqs["a1"][:], op=ALU.add)
    nc.vector.tensor_tensor(q2both[:, 1:2], sqs["rel"][:], sqs["a2"][:], op=ALU.add)
    q2tp = tpsum_pool.tile([128, NCORE], F32, tag="cp")
    nc.tensor.transpose(q2tp[:2, :B], q2both[:], ident[:B, :B])
    q2T_sb = small_pool.tile([2, B], BF16, tag="q2T_sb")
    nc.scalar.activation(q2T_sb[:], q2tp[:2, :B], AF.Copy, scale=-1.0)
    nc.sync.dma_start(relmov[100:101, 0:B], q2T_sb[0:1, :])
    nc.sync.dma_start(relmov[100:101, B : 2 * B], q2T_sb[1:2, :])

    # entities
    entn = small_pool.tile([128, 4 * E], F32, tag="entn")
    nc.sync.dma_start(
        entn[:].rearrange("p (c e) -> p c e", e=E),
        ent.ap().rearrange("(c p) e -> p c e", p=128),
    )
    ent2T = st["ent2T"]
    entsqT = small_pool.tile([100, NCORE], BF16, tag="entsqT")
    for c in range(4):
        tp = tpsum_pool.tile([128, NCORE], F32, tag="cp")
        nc.tensor.transpose(tp[:100, :128], entn[:, c * E : (c + 1) * E], ident[:])
        nc.scalar.activation(
            ent2T[:, c * 128 : (c + 1) * 128], tp[:100, :128], AF.Copy, scale=2.0
        )
        nc.scalar.activation(
            entsqT[:, c * 128 : (c + 1) * 128], tp[:100, :128], AF.Square
        )
    ones_col = small_pool.tile([100, 1], BF16, tag="ones_col")
    nc.gpsimd.memset(ones_col[:], 1.0)
    e2p = tpsum_pool.tile([128, NCORE], F32, tag="cp")
    nc.tensor.matmul(e2p[:1, :NCORE], ones_col[:], entsqT[:], start=True, stop=True)
    nc.scalar.activation(st["e2row"][:], e2p[:1, :NCORE], AF.Copy)


def _issue_fact_loads(nc, nat_pool, f_rel, f_a1, f_a2):
    nats = []
    for name, dram in (("rel", f_rel), ("a1", f_a1), ("a2", f_a2)):
        natt = nat_pool.tile([128, NCHUNK * E], F32, tag=f"nat_{name}")
        nats.append(natt)
    for g in range(GROUPS):
        for natt, dram in zip(nats, (f_rel, f_a1, f_a2)):
            nc.sync.dma_start(
                natt[:, g * GCH * E : (g + 1) * GCH * E].rearrange(
                    "p (c e) -> p c e", e=E
                ),
                dram.ap()[g * GCH * 128 : (g + 1) * GCH * 128, :].rearrange(
                    "(c p) e -> p c e", p=128
                ),
            )
    return nats


def _prologue_group(nc, st, nats, g, sq_pool, tpsum_pool, wpsum_pool):
    ident = st["ident"]
    f2cols = st["f2cols"]
    frelT, fa1T, fa2T = st["frelT"], st["fa1T"], st["fa2T"]
    W_all, Wrows = st["W_all"], st["Wrows"]
    relmov, a1mov, a2mov = st["relmov"], st["a1mov"], st["a2mov"]

    gs = slice(g * GCH, (g + 1) * GCH)
    ges = slice(g * GCH * E, (g + 1) * GCH * E)
    reds = []
    for i, natt in enumerate(nats):
        sq = sq_pool.tile([128, GCH * E], BF16, tag="sq")
        nc.scalar.activation(sq[:], natt[:, ges], AF.Square)
        dst = f2cols[:, gs] if i == 0 else None
        if dst is None:
            r = sq_pool.tile([128, GCH], F32, tag="fred")
            reds.append(r)
            dst = r[:]
        nc.vector.tensor_reduce(
            dst, sq[:].rearrange("p (c e) -> p c e", e=E),
            axis=mybir.AxisListType.X, op=ALU.add)
    nc.vector.tensor_tensor(f2cols[:, gs], f2cols[:, gs], reds[0][:], op=ALU.add)
    nc.vector.tensor_tensor(f2cols[:, gs], f2cols[:, gs], reds[1][:], op=ALU.add)

    for c in range(g * GCH, (g + 1) * GCH):
        cs = slice(c * 128, (c + 1) * 128)
        ces = slice(c * E, (c + 1) * E)
        for natt, dstT in ((nats[0], frelT), (nats[1], fa1T), (nats[2], fa2T)):
            tp = tpsum_pool.tile([128, NCORE], F32, tag="cp")
            nc.tensor.transpose(tp[:100, :128], natt[:, ces], ident[:])
            nc.vector.tensor_scalar(
                out=dstT[0:100, cs], in0=tp[:100, :128], scalar1=1.0,
                scalar2=None, op0=ALU.mult)

        wpt = wpsum_pool.tile([128, NCORE], F32, tag="pp")
        wp = wpt[:, :NSEC]
        nc.tensor.matmul(wp, frelT[:, cs], relmov[:], start=True, stop=False)
        nc.tensor.matmul(wp, fa1T[:, cs], a1mov[:], start=False, stop=False)
        nc.tensor.matmul(wp, fa2T[:, cs], a2mov[:], start=False, stop=True)
        wdst = W_all[:, c * NSEC : (c + 1) * NSEC]
        nc.vector.tensor_scalar(
            out=wdst, in0=wp, scalar1=f2cols[:, c : c + 1], scalar2=None,
            op0=ALU.subtract)
        for sc, anc in st["anchor"].items():
            lo = c * NSEC + sc * B
            nc.vector.tensor_scalar(
                out=st["W_del"][:, lo : lo + B],
                in0=W_all[:, lo : lo + B],
                scalar1=W_all[:, c * NSEC + anc : c * NSEC + anc + 1],
                scalar2=None, op0=ALU.subtract)


def _stage1(nc, add_engines, max_engines, slot, d_secs, h_secs, g_secs,
            p_secs, anchor, st, chunks, csb_pool, xall_pool, cpsum_pool,
            ppsum_pool, vcast):
    ND = len(d_secs)
    fa1T, fa2T = st["fa1T"], st["fa2T"]
    ent2T, W_all, Wrows, onehots = (
        st["ent2T"], st["W_all"], st["Wrows"], st["onehots"],
    )
    acc_all = st["acc_all"]
    fxT = {0: fa2T, 1: fa1T}  # score -> C stationary (sp: fact_arg2, po: fact_arg1)
    need_c = {0: False, 1: False}
    for sec in range(NSEC):
        if add_engines[sec] in ("v", "a", "g"):
            need_c[sec // B] = True
    for c in chunks:
        cs = slice(c * 128, (c + 1) * 128)

        cps = {}
        for s in (0, 1):
            if need_c[s]:
                cp = cpsum_pool.tile([128, NCORE], F32, tag="cp")
                nc.tensor.matmul(cp[:], fxT[s][:, cs], ent2T[:], start=True, stop=True)
                cps[s] = cp

        pps = {}
        for sec in p_secs:
            s = sec // B
            pp = ppsum_pool.tile([128, NCORE], F32, tag="pp")
            nc.tensor.matmul(pp[:], fxT[s][:, cs], ent2T[:], start=True, stop=False)
            nc.tensor.matmul(
                pp[:], Wrows[:, cs],
                onehots[:, sec * NCORE : (sec + 1) * NCORE],
                start=False, stop=True,
            )
            pps[sec] = pp

        xt = xall_pool.tile([128, NSEC * NCORE], BF16, tag="xall")
        anc_x = {}
        for sc, sec in anchor.items():
            sl = slot[sec]
            wcol = W_all[:, c * NSEC + sec : c * NSEC + sec + 1]
            xsec = xt[:, sl * NCORE : (sl + 1) * NCORE]
            nc.scalar.activation(xsec, cps[sc][:], AF.Identity, bias=wcol)
            anc_x[sc] = xsec
        for sec in range(NSEC):
            eng = add_engines[sec]
            if eng in ("p", "n"):
                continue
            s = sec // B
            sl = slot[sec]
            xsec = xt[:, sl * NCORE : (sl + 1) * NCORE]
            if eng == "a":
                wcol = W_all[:, c * NSEC + sec : c * NSEC + sec + 1]
                nc.scalar.activation(xsec, cps[s][:], AF.Identity, bias=wcol)
            elif eng == "v":
                dcol = st["W_del"][:, c * NSEC + sec : c * NSEC + sec + 1]
                nc.vector.tensor_scalar(
                    out=xsec, in0=anc_x[s], scalar1=dcol, scalar2=None,
                    op0=ALU.add)
            elif eng == "h":
                dcol = st["W_del"][:, c * NSEC + sec : c * NSEC + sec + 1]
                half = NCORE // 2
                nc.vector.tensor_scalar(
                    out=xsec[:, :half], in0=anc_x[s][:, :half], scalar1=dcol,
                    scalar2=None, op0=ALU.add)
                nc.gpsimd.tensor_scalar(
                    out=xsec[:, half:], in0=anc_x[s][:, half:], scalar1=dcol,
                    scalar2=None, op0=ALU.add)
            else:
                dcol = st["W_del"][:, c * NSEC + sec : c * NSEC + sec + 1]
                nc.gpsimd.tensor_scalar(
                    out=xsec, in0=anc_x[s], scalar1=dcol, scalar2=None,
                    op0=ALU.add)

        dw = ND * NCORE + (NCORE // 2 if h_secs else 0)
        if dw:
            nc.vector.tensor_tensor(
                acc_all[:, :dw], acc_all[:, :dw], xt[:, :dw], op=ALU.max)
        for sec in h_secs:
            sl = slot[sec]
            lo = sl * NCORE + NCORE // 2
            hi = (sl + 1) * NCORE
            nc.gpsimd.tensor_tensor(
                acc_all[:, lo:hi], acc_all[:, lo:hi], xt[:, lo:hi], op=ALU.max)
        for sec in g_secs:
            sl = slot[sec]
            asec = acc_all[:, sl * NCORE : (sl + 1) * NCORE]
            if sec in pps:
                nc.gpsimd.tensor_tensor(asec, asec, pps[sec][:], op=ALU.max)
            else:
                nc.gpsimd.tensor_tensor(
                    asec, asec, xt[:, sl * NCORE : (sl + 1) * NCORE], op=ALU.max)


_NC_CACHE = None


def get_nc():
    global _NC_CACHE
    if _NC_CACHE is None:
        _NC_CACHE = build_bass()
    return _NC_CACHE


def make_in_maps(rel, arg1, arg2, fact_rel, fact_arg1, fact_arg2, entity_embeddings):
    n_per = F // 8

    def pad_fact(m):
        out = np.full((FP, E), 10.0, dtype=np.float32)
        out[:F] = m
        return out

    frp, f1p, f2p = pad_fact(fact_rel), pad_fact(fact_arg1), pad_fact(fact_arg2)
    in_maps = []
    for c in range(8):
        ent_pad = np.zeros((NCORE, E), dtype=np.float32)
        ent_pad[:n_per] = entity_embeddings[c * n_per : (c + 1) * n_per]
        in_maps.append(
            {
                "f_rel": frp,
                "f_a1": f1p,
                "f_a2": f2p,
                "ent": ent_pad,
                "rel": np.ascontiguousarray(rel, dtype=np.float32),
                "a1": np.ascontiguousarray(arg1, dtype=np.float32),
                "a2": np.ascontiguousarray(arg2, dtype=np.float32),
            }
        )
    return in_maps


def assemble(results):
    n_per = F // 8
    order = section_order()
    inv = np.argsort(np.array(order))  # out row i holds section order[i]
    parts = [r["out"][inv].reshape(2, B, NCORE)[:, :, :n_per] for r in results]
    full = np.concatenate(parts, axis=2)
    return full[0].copy(), full[1].copy()


def kernel(rel, arg1, arg2, fact_rel, fact_arg1, fact_arg2, entity_embeddings):
    nc = get_nc()
    in_maps = make_in_maps(
        rel, arg1, arg2, fact_rel, fact_arg1, fact_arg2, entity_embeddings
    )
    res = bass_utils.run_bass_kernel_spmd(nc, in_maps, core_ids=list(range(8)))
    return assemble(res.results)


# revision 6
# speedup vs baseline: 6.0446x; 6.0446x over previous
"""NeuralKB retrieval kernel v2 for Trainium2 (Bass/Tile), 8-core SPMD.

Math (per score s in {sp, po}, batch b, entity n):
  d2min[s,b,n] = q2part[s,b] + e2[n] + min_f (f2[f] - 2*A[s,b,f] - 2*C[s,n,f])
  out[s,b,n]   = min(exp(-0.5 * (e2 - M)), 1),  M = max_f (2C + 2A - f2 - q2part)

Sharding: data-parallel over N (500 entities/core, padded to 512).
Layout: f on partitions (32 chunks of 128), n on free (512).

v2 vs v1:
- Loop-invariant work (fact transposes/casts, f2, W) hoisted out of the
  steady-state chunk loop; in the one-shot build it is interleaved per
  group of 8 chunks so stage-1 starts after ~1/4 of the input prep.
- W kept both as columns (W_all f32: per-partition scalars for lane adds)
  and as bf16 rows (Wrows: PE fusion below).
- 'p' sections: PE computes C + W in one PSUM tile per section via the
  C-part matmul plus a rank-16 matmul Wrows[:,chunk]^T @ onehot[sec]
  (one-hot column block selects that section's W row, broadcast over n);
  Pool then max-accumulates straight from PSUM. No lane add at all.
- Default stage-1 (bcast=True): 5 instructions per chunk — 2 C matmuls,
  2 broadcast-AP add-TTs (in0 = C PSUM tile broadcast over the 8 batch
  sections via a 0-stride dim, in1 = W columns broadcast over n), and one
  batched [128,16*512] DVE max. The tiny instruction stream matters more
  than engine 2x modes here: the axon/PJRT For_i replay pays ~us-scale
  per-instruction overhead, so the 544-instruction per-section variant
  (bcast=False, kept for reference) measures ~6.5x slower on HW despite
  a ~3x lower cost-model time.
- acc/out kept in 'slot' order (batched-DVE group contiguous);
  assemble() un-permutes on the host for free.
"""

import numpy as np

import concourse.bass as bass
import concourse.tile as tile
from concourse import bacc, mybir
from concourse import bass_utils
from concourse.masks import make_identity
from concourse.bass_isa import ReduceOp

F32 = mybir.dt.float32
BF16 = mybir.dt.bfloat16
AF = mybir.ActivationFunctionType
ALU = mybir.AluOpType

B = 8
E = 100
F = 4000
FP = 4096
NCHUNK = FP // 128  # 32
GROUPS = 4
GCH = NCHUNK // GROUPS  # 8
NCORE = 512
NSEC = 16  # (score, b) sections; id = score*8 + b
NEG_BIG = -3.0e38

# Per-section engine plan (index = score*8+b).
# add: 'p' PE-fused, 'v' DVE (bf16 cast), 'a' ACT (PSUM+bias), 'g' Pool (PSUM)
# max: 'd' batched DVE tt-max, 'g' per-section Pool tt-max ('p' adds => 'g')
# 'n' = anchor: ACT bias-add from PSUM writing bf16 xt; v/g sections add
# (w_sec - w_anchor) to the anchor's xt slice (bf16, no PSUM cast needed).
# Pool supports only tensor_scalar (no TensorTensor) on TRN2 HW, and cannot
# touch PSUM; so all max-accumulates are one batched DVE tensor_tensor and
# Pool only does bf16 delta-adds off each score's anchor section.
ADD_ENGINES = list("naaggggg" "naaagggg")
MAX_ENGINES = list("dddddddd" "dddddddd")


def section_order(max_engines=None):
    max_engines = max_engines or MAX_ENGINES
    d_secs = [s for s in range(NSEC) if max_engines[s] == "d"]
    h_secs = [s for s in range(NSEC) if max_engines[s] == "h"]
    g_secs = [s for s in range(NSEC) if max_engines[s] not in ("d", "h")]
    return d_secs + h_secs + g_secs


def build_bass(add_engines=None, max_engines=None, repeat=1,
               cbufs=5, pbufs=3, xbufs=3, csbufs=3, vcast="a", debug=False,
               bcast=True, bsrc="psum"):
    add_engines = add_engines or ADD_ENGINES
    max_engines = max_engines or MAX_ENGINES
    nc = bacc.Bacc("TRN2", target_bir_lowering=False, debug=False, num_devices=8)

    f_rel = nc.dram_tensor("f_rel", [FP, E], F32, kind="ExternalInput")
    f_a1 = nc.dram_tensor("f_a1", [FP, E], F32, kind="ExternalInput")
    f_a2 = nc.dram_tensor("f_a2", [FP, E], F32, kind="ExternalInput")
    ent = nc.dram_tensor("ent", [NCORE, E], F32, kind="ExternalInput")
    rel = nc.dram_tensor("rel", [B, E], F32, kind="ExternalInput")
    a1 = nc.dram_tensor("a1", [B, E], F32, kind="ExternalInput")
    a2 = nc.dram_tensor("a2", [B, E], F32, kind="ExternalInput")
    # out rows are in 'slot' order; assemble() un-permutes host-side
    out = nc.dram_tensor("out", [NSEC, NCORE], F32, kind="ExternalOutput")
    dbg = None
    if debug:
        dbg = {
            "dbg_acc": nc.dram_tensor("dbg_acc", [128, NSEC * NCORE], BF16, kind="ExternalOutput"),
            "dbg_m16": nc.dram_tensor("dbg_m16", [NSEC, NCORE], BF16, kind="ExternalOutput"),
            "dbg_e2": nc.dram_tensor("dbg_e2", [1, NCORE], F32, kind="ExternalOutput"),
            "dbg_w": nc.dram_tensor("dbg_w", [128, NCHUNK * NSEC], F32, kind="ExternalOutput"),
            "dbg_wd": nc.dram_tensor("dbg_wd", [128, NCHUNK * NSEC], F32, kind="ExternalOutput"),
            "dbg_f2": nc.dram_tensor("dbg_f2", [128, NCHUNK], F32, kind="ExternalOutput"),
        }

    with tile.TileContext(nc) as tc:
        _kernel_body(nc, tc, f_rel, f_a1, f_a2, ent, rel, a1, a2, out,
                     add_engines, max_engines, repeat, cbufs, pbufs, xbufs,
                     csbufs, vcast, dbg, bcast, bsrc)
    nc.compile()
    return nc


def _kernel_body(nc, tc, f_rel, f_a1, f_a2, ent, rel, a1, a2, out,
                 add_engines, max_engines, repeat, cbufs, pbufs, xbufs,
                 csbufs, vcast, dbg=None, bcast=False, bsrc="psum"):
    secs = list(range(NSEC))
    order = section_order(max_engines)
    d_secs = [s for s in secs if max_engines[s] == "d"]
    h_secs = [s for s in secs if max_engines[s] == "h"]
    assert len(h_secs) <= 1, "at most one half-split max section"
    g_secs = [s for s in secs if max_engines[s] not in ("d", "h")]
    slot = {sec: i for i, sec in enumerate(order)}
    p_secs = [s for s in secs if add_engines[s] == "p"]
    for s in p_secs:
        assert max_engines[s] == "g", "p-sections read PSUM; use Pool max"
    anchor = {}
    for s in secs:
        if add_engines[s] == "n":
            sc = s // B
            assert sc not in anchor, "one anchor per score"
            anchor[sc] = s
    for s in secs:
        if add_engines[s] in ("v", "g"):
            assert s // B in anchor, "delta sections need an anchor in their score"

    with (
        tc.tile_pool(name="const", bufs=1) as const_pool,
        tc.tile_pool(name="factT", bufs=1) as factT_pool,
        tc.tile_pool(name="acc", bufs=1) as acc_pool,
        tc.tile_pool(name="small", bufs=1) as small_pool,
        tc.tile_pool(name="fin", bufs=1) as fin_pool,
        tc.tile_pool(name="nat", bufs=1) as nat_pool,
        tc.tile_pool(name="sq", bufs=3) as sq_pool,
    ):
        st = {}  # shared tiles
        st["anchor"] = anchor
        st["ident"] = const_pool.tile([128, 128], F32, name="ident")
        make_identity(nc, st["ident"][:])

        frelT = factT_pool.tile([101, FP], BF16, tag="frelT")
        st["frelT"] = frelT
        st["fa1T"] = factT_pool.tile([100, FP], BF16, name="fa1T")
        st["fa2T"] = factT_pool.tile([100, FP], BF16, name="fa2T")
        onesrowF = small_pool.tile([1, FP], BF16, tag="onesrowF")
        nc.gpsimd.memset(onesrowF[:], 1.0)
        nc.sync.dma_start(frelT[100:101, :], onesrowF[:])

        st["W_all"] = const_pool.tile([128, NCHUNK * NSEC], F32, name="W_all")
        st["W_del"] = const_pool.tile([128, NCHUNK * NSEC], F32, name="W_del")
        st["Wrows"] = const_pool.tile([NSEC, NCHUNK * 128], BF16, name="Wrows")
        onehots = const_pool.tile([NSEC, NSEC * NCORE], BF16, tag="onehots")
        st["onehots"] = onehots
        nc.gpsimd.memset(onehots[:], 0.0)
        for sec in p_secs:
            nc.sync.dma_start(
                onehots[sec : sec + 1, sec * NCORE : (sec + 1) * NCORE],
                onesrowF[0:1, :NCORE],
            )
        st["ent2T"] = const_pool.tile([100, NCORE], BF16, name="ent2T")
        st["e2row"] = small_pool.tile([1, NCORE], F32, name="e2row")
        acc_all = acc_pool.tile([128, NSEC * NCORE], BF16, tag="acc_all")
        st["acc_all"] = acc_all
        nc.gpsimd.memset(acc_all[:], NEG_BIG)
        st["f2cols"] = small_pool.tile([128, NCHUNK], F32, name="f2cols")

        with (
            tc.tile_pool(name="csb", bufs=csbufs) as csb_pool,
            tc.tile_pool(name="xall", bufs=xbufs) as xall_pool,
            tc.tile_pool(name="cpsum", bufs=cbufs, space="PSUM") as cpsum_pool,
            tc.tile_pool(name="ppsum", bufs=pbufs, space="PSUM") as ppsum_pool,
        ):
            import contextlib

            _prologue_head(nc, st, ent, rel, a1, a2, cpsum_pool, small_pool)
            nats = _issue_fact_loads(nc, nat_pool, f_rel, f_a1, f_a2)

            def stage1(chunks):
                if bcast:
                    _stage1_bcast(nc, st, chunks, csb_pool, xall_pool,
                                  cpsum_pool, bsrc)
                else:
                    _stage1(nc, add_engines, max_engines, slot, d_secs,
                            h_secs, g_secs, p_secs, anchor, st, chunks,
                            csb_pool, xall_pool, cpsum_pool, ppsum_pool,
                            vcast)

            if repeat == 1:
                for g in range(GROUPS):
                    _prologue_group(nc, st, nats, g, sq_pool, cpsum_pool,
                                    ppsum_pool)
                    stage1(range(g * GCH, (g + 1) * GCH))
            else:
                for g in range(GROUPS):
                    _prologue_group(nc, st, nats, g, sq_pool, cpsum_pool,
                                    ppsum_pool)
                if isinstance(repeat, str):  # "uN" = static unroll N copies
                    for _ in range(int(repeat[1:])):
                        stage1(range(NCHUNK))
                elif isinstance(repeat, tuple):  # (R, U): For_i(R) x U unrolled
                    rr, uu = repeat
                    with tc.For_i(0, rr, 1):
                        for _ in range(uu):
                            stage1(range(NCHUNK))
                else:
                    with tc.For_i(0, repeat, 1):
                        stage1(range(NCHUNK))

        # ---------------- finals (slot order) ----------------------------
        if dbg is not None:
            nc.sync.dma_start(dbg["dbg_acc"].ap(), acc_all[:])
            nc.sync.dma_start(dbg["dbg_e2"].ap(), st["e2row"][:])
            nc.sync.dma_start(dbg["dbg_w"].ap(), st["W_all"][:])
            nc.sync.dma_start(dbg["dbg_wd"].ap(), st["W_del"][:])
            nc.sync.dma_start(dbg["dbg_f2"].ap(), st["f2cols"][:])
        accmax = acc_pool.tile([128, NSEC * NCORE], BF16, tag="accmax")
        m16 = fin_pool.tile([NSEC, NCORE], BF16, tag="m16")
        nc.gpsimd.partition_all_reduce(accmax[:], acc_all[:], 128, ReduceOp.max)
        for i in range(NSEC):
            nc.sync.dma_start(
                m16[i : i + 1, :], accmax[0:1, i * NCORE : (i + 1) * NCORE]
            )
        if dbg is not None:
            nc.sync.dma_start(dbg["dbg_m16"].ap(), m16[:])
        e2rep = fin_pool.tile([NSEC, NCORE], F32, tag="e2rep")
        nc.gpsimd.partition_broadcast(e2rep[:], st["e2row"][:])
        sub16 = fin_pool.tile([NSEC, NCORE], F32, tag="sub16")
        nc.vector.tensor_tensor(sub16[:], e2rep[:], m16[:], op=ALU.subtract)
        exp16 = fin_pool.tile([NSEC, NCORE], F32, tag="exp16")
        nc.scalar.activation(exp16[:], sub16[:], AF.Exp, scale=-0.5)
        out16 = fin_pool.tile([NSEC, NCORE], F32, tag="out16")
        nc.vector.tensor_scalar(
            out=out16[:], in0=exp16[:], scalar1=1.0, scalar2=None, op0=ALU.min
        )
        nc.sync.dma_start(out.ap(), out16[:])


def _prologue_head(nc, st, ent, rel, a1, a2, tpsum_pool, small_pool):
    ident = st["ident"]
    relmov = small_pool.tile([101, 2 * B], BF16, tag="relmov")
    a1mov = small_pool.tile([100, 2 * B], BF16, tag="a1mov")
    a2mov = small_pool.tile([100, 2 * B], BF16, tag="a2mov")
    st["relmov"], st["a1mov"], st["a2mov"] = relmov, a1mov, a2mov
    nc.gpsimd.memset(a1mov[:, B : 2 * B], 0.0)
    nc.gpsimd.memset(a2mov[:, 0:B], 0.0)

    qn = {}
    for name, dram in (("rel", rel), ("a1", a1), ("a2", a2)):
        qt = small_pool.tile([B, E], F32, tag=f"q_{name}")
        nc.sync.dma_start(qt[:], dram.ap())
        qn[name] = qt
    for name, dsts in (
        ("rel", ((relmov, 0), (relmov, B))),
        ("a1", ((a1mov, 0),)),
        ("a2", ((a2mov, B),)),
    ):
        tp = tpsum_pool.tile([128, NCORE], F32, tag="cp")
        nc.tensor.transpose(tp[:E, :B], qn[name][:], ident[:B, :B])
        for dst, coff in dsts:
            nc.scalar.activation(
                dst[0:100, coff : coff + B], tp[:E, :B], AF.Copy, scale=2.0
            )
    sqs = {}
    for name in ("rel", "a1", "a2"):
        sq = small_pool.tile([B, E], F32, tag=f"qsq_{name}")
        nc.scalar.activation(sq[:], qn[name][:], AF.Square)
        r = small_pool.tile([B, 1], F32, tag=f"qr_{name}")
        nc.vector.tensor_reduce(r[:], sq[:], axis=mybir.AxisListType.X, op=ALU.add)
        sqs[name] = r
    q2both = small_pool.tile([B, 2], F32, tag="q2both")
    nc.vector.tensor_tensor(q2both[:, 0:1], sqs["rel"][:], sqs["a1"][:], op=ALU.add)
    nc.vector.tensor_tensor(q2both[:, 1:2], sqs["rel"][:], sqs["a2"][:], op=ALU.add)
    q2tp = tpsum_pool.tile([128, NCORE], F32, tag="cp")
    nc.tensor.transpose(q2tp[:2, :B], q2both[:], ident[:B, :B])
    q2T_sb = small_pool.tile([2, B], BF16, tag="q2T_sb")
    nc.scalar.activation(q2T_sb[:], q2tp[:2, :B], AF.Copy, scale=-1.0)
    nc.sync.dma_start(relmov[100:101, 0:B], q2T_sb[0:1, :])
    nc.sync.dma_start(relmov[100:101, B : 2 * B], q2T_sb[1:2, :])

    # entities
    entn = small_pool.tile([128, 4 * E], F32, tag="entn")
    nc.sync.dma_start(
        entn[:].rearrange("p (c e) -> p c e", e=E),
        ent.ap().rearrange("(c p) e -> p c e", p=128),
    )
    ent2T = st["ent2T"]
    entsqT = small_pool.tile([100, NCORE], BF16, tag="entsqT")
    for c in range(4):
        tp = tpsum_pool.tile([128, NCORE], F32, tag="cp")
        nc.tensor.transpose(tp[:100, :128], entn[:, c * E : (c + 1) * E], ident[:])
        nc.scalar.activation(
            ent2T[:, c * 128 : (c + 1) * 128], tp[:100, :128], AF.Copy, scale=2.0
        )
        nc.scalar.activation(
            entsqT[:, c * 128 : (c + 1) * 128], tp[:100, :128], AF.Square
        )
    ones_col = small_pool.tile([100, 1], BF16, tag="ones_col")
    nc.gpsimd.memset(ones_col[:], 1.0)
    e2p = tpsum_pool.tile([128, NCORE], F32, tag="cp")
    nc.tensor.matmul(e2p[:1, :NCORE], ones_col[:], entsqT[:], start=True, stop=True)
    nc.scalar.activation(st["e2row"][:], e2p[:1, :NCORE], AF.Copy)


def _issue_fact_loads(nc, nat_pool, f_rel, f_a1, f_a2):
    nats = []
    for name, dram in (("rel", f_rel), ("a1", f_a1), ("a2", f_a2)):
        natt = nat_pool.tile([128, NCHUNK * E], F32, tag=f"nat_{name}")
        nats.append(natt)
    for g in range(GROUPS):
        for natt, dram in zip(nats, (f_rel, f_a1, f_a2)):
            nc.sync.dma_start(
                natt[:, g * GCH * E : (g + 1) * GCH * E].rearrange(
                    "p (c e) -> p c e", e=E
                ),
                dram.ap()[g * GCH * 128 : (g + 1) * GCH * 128, :].rearrange(
                    "(c p) e -> p c e", p=128
                ),
            )
    return nats


def _prologue_group(nc, st, nats, g, sq_pool, tpsum_pool, wpsum_pool):
    ident = st["ident"]
    f2cols = st["f2cols"]
    frelT, fa1T, fa2T = st["frelT"], st["fa1T"], st["fa2T"]
    W_all, Wrows = st["W_all"], st["Wrows"]
    relmov, a1mov, a2mov = st["relmov"], st["a1mov"], st["a2mov"]

    gs = slice(g * GCH, (g + 1) * GCH)
    ges = slice(g * GCH * E, (g + 1) * GCH * E)
    reds = []
    for i, natt in enumerate(nats):
        sq = sq_pool.tile([128, GCH * E], BF16, tag="sq")
        nc.scalar.activation(sq[:], natt[:, ges], AF.Square)
        dst = f2cols[:, gs] if i == 0 else None
        if dst is None:
            r = sq_pool.tile([128, GCH], F32, tag="fred")
            reds.append(r)
            dst = r[:]
        nc.vector.tensor_reduce(
            dst, sq[:].rearrange("p (c e) -> p c e", e=E),
            axis=mybir.AxisListType.X, op=ALU.add)
    nc.vector.tensor_tensor(f2cols[:, gs], f2cols[:, gs], reds[0][:], op=ALU.add)
    nc.vector.tensor_tensor(f2cols[:, gs], f2cols[:, gs], reds[1][:], op=ALU.add)

    for c in range(g * GCH, (g + 1) * GCH):
        cs = slice(c * 128, (c + 1) * 128)
        ces = slice(c * E, (c + 1) * E)
        for natt, dstT in ((nats[0], frelT), (nats[1], fa1T), (nats[2], fa2T)):
            tp = tpsum_pool.tile([128, NCORE], F32, tag="cp")
            nc.tensor.transpose(tp[:100, :128], natt[:, ces], ident[:])
            nc.vector.tensor_scalar(
                out=dstT[0:100, cs], in0=tp[:100, :128], scalar1=1.0,
                scalar2=None, op0=ALU.mult)

        wpt = wpsum_pool.tile([128, NCORE], F32, tag="pp")
        wp = wpt[:, :NSEC]
        nc.tensor.matmul(wp, frelT[:, cs], relmov[:], start=True, stop=False)
        nc.tensor.matmul(wp, fa1T[:, cs], a1mov[:], start=False, stop=False)
        nc.tensor.matmul(wp, fa2T[:, cs], a2mov[:], start=False, stop=True)
        wdst = W_all[:, c * NSEC : (c + 1) * NSEC]
        nc.vector.tensor_scalar(
            out=wdst, in0=wp, scalar1=f2cols[:, c : c + 1], scalar2=None,
            op0=ALU.subtract)
        for sc, anc in st["anchor"].items():
            lo = c * NSEC + sc * B
            nc.vector.tensor_scalar(
                out=st["W_del"][:, lo : lo + B],
                in0=W_all[:, lo : lo + B],
                scalar1=W_all[:, c * NSEC + anc : c * NSEC + anc + 1],
                scalar2=None, op0=ALU.subtract)


def _stage1(nc, add_engines, max_engines, slot, d_secs, h_secs, g_secs,
            p_secs, anchor, st, chunks, csb_pool, xall_pool, cpsum_pool,
            ppsum_pool, vcast):
    ND = len(d_secs)
    fa1T, fa2T = st["fa1T"], st["fa2T"]
    ent2T, W_all, Wrows, onehots = (
        st["ent2T"], st["W_all"], st["Wrows"], st["onehots"],
    )
    acc_all = st["acc_all"]
    fxT = {0: fa2T, 1: fa1T}  # score -> C stationary (sp: fact_arg2, po: fact_arg1)
    need_c = {0: False, 1: False}
    for sec in range(NSEC):
        if add_engines[sec] in ("v", "a", "g"):
            need_c[sec // B] = True
    for c in chunks:
        cs = slice(c * 128, (c + 1) * 128)

        cps = {}
        for s in (0, 1):
            if need_c[s]:
                cp = cpsum_pool.tile([128, NCORE], F32, tag="cp")
                nc.tensor.matmul(cp[:], fxT[s][:, cs], ent2T[:], start=True, stop=True)
                cps[s] = cp

        pps = {}
        for sec in p_secs:
            s = sec // B
            pp = ppsum_pool.tile([128, NCORE], F32, tag="pp")
            nc.tensor.matmul(pp[:], fxT[s][:, cs], ent2T[:], start=True, stop=False)
            nc.tensor.matmul(
                pp[:], Wrows[:, cs],
                onehots[:, sec * NCORE : (sec + 1) * NCORE],
                start=False, stop=True,
            )
            pps[sec] = pp

        xt = xall_pool.tile([128, NSEC * NCORE], BF16, tag="xall")
        anc_x = {}
        for sc, sec in anchor.items():
            sl = slot[sec]
            wcol = W_all[:, c * NSEC + sec : c * NSEC + sec + 1]
            xsec = xt[:, sl * NCORE : (sl + 1) * NCORE]
            nc.scalar.activation(xsec, cps[sc][:], AF.Identity, bias=wcol)
            anc_x[sc] = xsec
        for sec in range(NSEC):
            eng = add_engines[sec]
            if eng in ("p", "n"):
                continue
            s = sec // B
            sl = slot[sec]
            xsec = xt[:, sl * NCORE : (sl + 1) * NCORE]
            if eng == "a":
                wcol = W_all[:, c * NSEC + sec : c * NSEC + sec + 1]
                nc.scalar.activation(xsec, cps[s][:], AF.Identity, bias=wcol)
            elif eng == "v":
                dcol = st["W_del"][:, c * NSEC + sec : c * NSEC + sec + 1]
                nc.vector.tensor_scalar(
                    out=xsec, in0=anc_x[s], scalar1=dcol, scalar2=None,
                    op0=ALU.add)
            elif eng == "h":
                dcol = st["W_del"][:, c * NSEC + sec : c * NSEC + sec + 1]
                half = NCORE // 2
                nc.vector.tensor_scalar(
                    out=xsec[:, :half], in0=anc_x[s][:, :half], scalar1=dcol,
                    scalar2=None, op0=ALU.add)
                nc.gpsimd.tensor_scalar(
                    out=xsec[:, half:], in0=anc_x[s][:, half:], scalar1=dcol,
                    scalar2=None, op0=ALU.add)
            else:
                dcol = st["W_del"][:, c * NSEC + sec : c * NSEC + sec + 1]
                nc.gpsimd.tensor_scalar(
                    out=xsec, in0=anc_x[s], scalar1=dcol, scalar2=None,
                    op0=ALU.add)

        dw = ND * NCORE + (NCORE // 2 if h_secs else 0)
        if dw:
            nc.vector.tensor_tensor(
                acc_all[:, :dw], acc_all[:, :dw], xt[:, :dw], op=ALU.max)
        for sec in h_secs:
            sl = slot[sec]
            lo = sl * NCORE + NCORE // 2
            hi = (sl + 1) * NCORE
            nc.gpsimd.tensor_tensor(
                acc_all[:, lo:hi], acc_all[:, lo:hi], xt[:, lo:hi], op=ALU.max)
        for sec in g_secs:
            sl = slot[sec]
            asec = acc_all[:, sl * NCORE : (sl + 1) * NCORE]
            if sec in pps:
                nc.gpsimd.tensor_tensor(asec, asec, pps[sec][:], op=ALU.max)
            else:
                nc.gpsimd.tensor_tensor(
                    asec, asec, xt[:, sl * NCORE : (sl + 1) * NCORE], op=ALU.max)


def _stage1_bcast(nc, st, chunks, csb_pool, xall_pool, cpsum_pool, bsrc):
    """Broadcast-AP stage-1.

    bsrc="psum":  5 instr/chunk: 2 C matmuls + 2 broadcast add-TTs (PSUM in)
                  + 1 max-TT.
    bsrc="psum1": 4 instr/chunk: both C matmuls land in one double-wide PSUM
                  tile; ONE 4D broadcast add-TT covers all 16 sections.
    bsrc="a":     7 instr/chunk: ACT casts then bf16 2x broadcast add-TTs.
    X[p, s*512+n] = C_score[p, n] + W[p, s] via 0-stride broadcast APs.
    Requires slot order == section order (all-'d' maxes).
    """
    fa1T, fa2T = st["fa1T"], st["fa2T"]
    ent2T, W_all, acc_all = st["ent2T"], st["W_all"], st["acc_all"]
    fxT = {0: fa2T, 1: fa1T}
    for c in chunks:
        cs = slice(c * 128, (c + 1) * 128)
        xt = xall_pool.tile([128, NSEC * NCORE], BF16, tag="xall")
        if bsrc == "psum1":
            cp = cpsum_pool.tile([128, 2 * NCORE], F32, tag="cp")
            for s in (0, 1):
                nc.tensor.matmul(
                    cp[:, s * NCORE : (s + 1) * NCORE], fxT[s][:, cs], ent2T[:],
                    start=True, stop=True)
            c_b = cp[:].rearrange("p (c2 o n) -> p c2 o n", c2=2, o=1).broadcast_to(
                (128, 2, B, NCORE))
            w_b = (
                W_all[:, c * NSEC : (c + 1) * NSEC]
                .rearrange("p (c2 s o) -> p c2 s o", c2=2, o=1)
                .broadcast_to((128, 2, B, NCORE))
            )
            nc.vector.tensor_tensor(
                xt[:].rearrange("p (c2 s n) -> p c2 s n", c2=2, s=B),
                c_b, w_b, op=ALU.add)
        else:
            cps = {}
            for s in (0, 1):
                cp = cpsum_pool.tile([128, NCORE], F32, tag="cp")
                nc.tensor.matmul(cp[:], fxT[s][:, cs], ent2T[:], start=True, stop=True)
                cps[s] = cp
            for s in (0, 1):
                if bsrc == "psum":
                    cin = cps[s][:]
                else:
                    csb = csb_pool.tile([128, NCORE], BF16, tag="csb")
                    nc.scalar.activation(csb[:], cps[s][:], AF.Copy)
                    cin = csb[:]
                c_b = cin.rearrange("p (o n) -> p o n", o=1).broadcast_to(
                    (128, B, NCORE))
                w_b = (
                    W_all[:, c * NSEC + s * B : c * NSEC + (s + 1) * B]
                    .rearrange("p (s o) -> p s o", o=1)
                    .broadcast_to((128, B, NCORE))
                )
                nc.vector.tensor_tensor(
                    xt[:, s * B * NCORE : (s + 1) * B * NCORE].rearrange(
                        "p (s n) -> p s n", s=B
                    ),
                    c_b, w_b, op=ALU.add,
                )
        nc.vector.tensor_tensor(acc_all[:], acc_all[:], xt[:], op=ALU.max)


_NC_CACHE = None


def get_nc():
    global _NC_CACHE
    if _NC_CACHE is None:
        _NC_CACHE = build_bass()
    return _NC_CACHE


def make_in_maps(rel, arg1, arg2, fact_rel, fact_arg1, fact_arg2, entity_embeddings):
    n_per = F // 8

    def pad_fact(m):
        out = np.full((FP, E), 10.0, dtype=np.float32)
        out[:F] = m
        return out

    frp, f1p, f2p = pad_fact(fact_rel), pad_fact(fact_arg1), pad_fact(fact_arg2)
    in_maps = []
    for c in range(8):
        ent_pad = np.zeros((NCORE, E), dtype=np.float32)
        ent_pad[:n_per] = entity_embeddings[c * n_per : (c + 1) * n_per]
        in_maps.append(
            {
                "f_rel": frp,
                "f_a1": f1p,
                "f_a2": f2p,
                "ent": ent_pad,
                "rel": np.ascontiguousarray(rel, dtype=np.float32),
                "a1": np.ascontiguousarray(arg1, dtype=np.float32),
                "a2": np.ascontiguousarray(arg2, dtype=np.float32),
            }
        )
    return in_maps


def assemble(results):
    n_per = F // 8
    order = section_order()
    inv = np.argsort(np.array(order))  # out row i holds section order[i]
    parts = [r["out"][inv].reshape(2, B, NCORE)[:, :, :n_per] for r in results]
    full = np.concatenate(parts, axis=2)
    return full[0].copy(), full[1].copy()


def kernel(rel, arg1, arg2, fact_rel, fact_arg1, fact_arg2, entity_embeddings):
    nc = get_nc()
    in_maps = make_in_maps(
        rel, arg1, arg2, fact_rel, fact_arg1, fact_arg2, entity_embeddings
    )
    res = bass_utils.run_bass_kernel_spmd(nc, in_maps, core_ids=list(range(8)))
    return assemble(res.results)
